# revision 3
# baseline (speedup 1.0000x reference)
"""CFConv (SchNet continuous-filter conv) Trainium2 Bass kernel, 8-core SPMD.

Reference computation:
    f    = x @ W_in                        # (40000, 128)
    f_j  = f[idx_j]                        # (640000, 128) gather
    wf   = w_ij * f_j                      # elementwise
    conv = segment_sum(wf, seg_i, 40000)   # seg_i sorted
    out  = conv @ W_out + b_out

Sharding: seg_i is sorted, so atoms are sharded into 8 contiguous ranges of
5000 and each core gets the contiguous run of edges whose seg_i falls in its
range (found with searchsorted on the host).  No collective is needed: each
core owns its 5000 output rows.

Per core the edge run is re-bucketed by 128-atom sub-window of seg_i, each
sub-window padded to a fixed chunk capacity so all 8 cores run one identical
SPMD program.  Because dma_gather indices are int16, each sub-window's edges
are split by idx_j half (< 20000 vs >= 20000) into leading / trailing chunk
groups and gathered by two dma_gather calls (the second from an offset AP of
the f scratch).  On device:

  phase 1: f = x @ W_in into an HBM scratch (x passed pre-transposed so x
           tiles serve directly as matmul lhsT).
  phase 2: per sub-window: DMA the wf-ready w tile, dma_gather f[idx_j] rows,
           DVE multiply, build the one-hot segment matrix with an is_equal
           compare against an iota tile, and matmul-accumulate
           convT[feat, atom] in PSUM (contraction over the edge partition
           axis).  Per 1024-atom window: fac2out matmul with W_out + bias.
"""

import numpy as np

import concourse.bass as bass
import concourse.mybir as mybir
from concourse import bacc
from concourse.tile import TileContext

P = 128
NA = 40000          # atoms
NE = 640000         # edges
D = 128             # feature dim (FAN_IN == NFM == FAN_OUT)
HALF = NA // 2      # dma_gather int16 index limit workaround
NCORES = 8
APC = NA // NCORES  # atoms per core = 5000
WIN = 1024          # atoms per PSUM window (2 banks)
SUB = 128           # atoms per sub-window (one matmul N slice)
NSW = (APC + SUB - 1) // SUB   # sub-windows per core = 40

F32 = mybir.dt.float32
I16 = mybir.dt.int16


def build_program(cap_lo: int, cap_hi: int):
    """One SPMD program, identical across cores."""
    nc = bacc.Bacc(None, target_bir_lowering=False, debug=False)
    cap = cap_lo + cap_hi
    esw = cap * P

    xT_h = nc.dram_tensor("xT", [P, NA], F32, kind="ExternalInput")
    wdev_h = nc.dram_tensor("wdev", [NSW, P, esw], F32, kind="ExternalInput")
    segw_h = nc.dram_tensor("segw", [P, NSW * cap], F32, kind="ExternalInput")
    idx16_h = nc.dram_tensor("idx16", [P, NSW * cap * 8], I16, kind="ExternalInput")
    iota_h = nc.dram_tensor("iota", [P, esw], F32, kind="ExternalInput")
    win_h = nc.dram_tensor("Win", [P, P], F32, kind="ExternalInput")
    wout_h = nc.dram_tensor("Wout", [P, P], F32, kind="ExternalInput")
    bias_h = nc.dram_tensor("bias", [P, P], F32, kind="ExternalInput")
    out_h = nc.dram_tensor("out", [APC, D], F32, kind="ExternalOutput")
    f_h = nc.dram_tensor("fscratch", [NA, D], F32, kind="Internal")

    with TileContext(nc) as tc:
        with tc.tile_pool(name="const", bufs=1) as const:
            win_t = const.tile([P, P], F32)
            nc.sync.dma_start(win_t[:], win_h[:, :])
            wout_t = const.tile([P, P], F32)
            nc.sync.dma_start(wout_t[:], wout_h[:, :])
            bias_t = const.tile([P, P], F32)
            nc.sync.dma_start(bias_t[:], bias_h[:, :])
            iota_t = const.tile([P, esw], F32)
            nc.sync.dma_start(iota_t[:], iota_h[:, :])
            segw_t = const.tile([P, NSW * cap], F32)
            nc.sync.dma_start(segw_t[:], segw_h[:, :])
            idx16_t = const.tile([P, NSW * cap * 8], I16)
            nc.sync.dma_start(idx16_t[:], idx16_h[:, :])

            # ---- phase 1: f = x @ W_in -> HBM scratch ----
            with (
                tc.tile_pool(name="xp", bufs=3) as xp,
                tc.tile_pool(name="fp", bufs=3) as fp,
                tc.tile_pool(name="ps1", bufs=2, space="PSUM") as ps1,
            ):
                a0 = 0
                while a0 < NA:
                    an = min(512, NA - a0)
                    xt = xp.tile([P, 512], F32)
                    nc.sync.dma_start(xt[:, :an], xT_h[:, a0 : a0 + an])
                    fps = ps1.tile([P, 4, P], F32)
                    nt = (an + P - 1) // P
                    for i in range(nt):
                        m = min(P, an - i * P)
                        nc.tensor.matmul(
                            fps[:m, i, :],
                            lhsT=xt[:, i * P : i * P + m],
                            rhs=win_t[:],
                            start=True,
                            stop=True,
                        )
                    fsb = fp.tile([P, 4, P], F32)
                    if an % P == 0:
                        nc.vector.tensor_copy(fsb[:, :nt, :], fps[:, :nt, :])
                        nc.sync.dma_start(
                            f_h[a0 : a0 + an, :].rearrange("(i p) e -> p i e", p=P),
                            fsb[:, :nt, :],
                        )
                    else:
                        nc.vector.tensor_copy(fsb[:an, 0, :], fps[:an, 0, :])
                        nc.sync.dma_start(f_h[a0 : a0 + an, :], fsb[:an, 0, :])
                    a0 += an

            # ---- phase 2: gather, multiply, segment-sum, fac2out ----
            with (
                tc.tile_pool(name="wp", bufs=3) as wp,
                tc.tile_pool(name="fjp", bufs=3) as fjp,
                tc.tile_pool(name="ohp", bufs=2) as ohp,
                tc.tile_pool(name="cvp", bufs=2) as cvp,
                tc.tile_pool(name="owp", bufs=2) as owp,
                tc.tile_pool(name="ps2", bufs=2, space="PSUM") as ps2,
                tc.tile_pool(name="ps3", bufs=2, space="PSUM") as ps3,
            ):
                psT = None
                for s in range(NSW):
                    w_i, sl = divmod(s, WIN // SUB)
                    wt = wp.tile([P, cap, P], F32)
                    nc.sync.dma_start(
                        wt[:], wdev_h[s].rearrange("p (c e) -> p c e", e=P)
                    )
                    fj = fjp.tile([P, cap, P], F32)
                    ib = s * cap * 8
                    # single_packet=False: >1008 idxs exceeds the 64-desc
                    # packet ceiling (HW-verified INTERNAL error otherwise)
                    nc.gpsimd.dma_gather(
                        fj[:, 0:cap_lo, :],
                        f_h[0:HALF, :],
                        idx16_t[:, ib : ib + cap_lo * 8],
                        cap_lo * P,
                        cap_lo * P,
                        D,
                        single_packet=False,
                    )
                    nc.gpsimd.dma_gather(
                        fj[:, cap_lo:cap, :],
                        f_h[HALF:NA, :],
                        idx16_t[:, ib + cap_lo * 8 : ib + cap * 8],
                        cap_hi * P,
                        cap_hi * P,
                        D,
                        single_packet=False,
                    )
                    nc.vector.tensor_mul(wt[:], wt[:], fj[:])
                    oh = ohp.tile([P, cap, P], F32)
                    nc.vector.tensor_tensor(
                        out=oh[:],
                        in0=segw_t[:, s * cap : (s + 1) * cap]
                        .unsqueeze(2)
                        .to_broadcast([P, cap, P]),
                        in1=iota_t[:].rearrange("p (c e) -> p c e", e=P),
                        op=mybir.AluOpType.is_equal,
                    )
                    if sl == 0:
                        psT = ps2.tile([P, WIN], F32)
                    for ch in range(cap):
                        nc.tensor.matmul(
                            psT[:, sl * SUB : (sl + 1) * SUB],
                            lhsT=wt[:, ch, :],
                            rhs=oh[:, ch, :],
                            start=(ch == 0),
                            stop=(ch == cap - 1),
                        )
                    if sl == WIN // SUB - 1 or s == NSW - 1:
                        wa0 = w_i * WIN
                        wan = min(WIN, APC - wa0)
                        cvt = cvp.tile([P, WIN], F32)
                        nc.vector.tensor_copy(cvt[:], psT[:])
                        ow = owp.tile([P, WIN // SUB, P], F32)
                        nblk = (wan + P - 1) // P
                        for b in range(nblk):
                            bm = min(P, wan - b * P)
                            ops3 = ps3.tile([P, P], F32)
                            nc.tensor.matmul(
                                ops3[:bm, :],
                                lhsT=cvt[:, b * P : b * P + bm],
                                rhs=wout_t[:],
                                start=True,
                                stop=True,
                            )
                            nc.vector.tensor_add(
                                ow[:bm, b, :], ops3[:bm, :], bias_t[:bm, :]
                            )
                        nfull = wan // P
                        if nfull:
                            nc.sync.dma_start(
                                out_h[wa0 : wa0 + nfull * P, :].rearrange(
                                    "(b p) e -> p b e", p=P
                                ),
                                ow[:, :nfull, :],
                            )
                        rem = wan - nfull * P
                        if rem:
                            nc.sync.dma_start(
                                out_h[wa0 + nfull * P : wa0 + wan, :],
                                ow[:rem, nfull, :],
                            )
    return nc


def _wrap_idx(idx):
    """idx [n] (n % 128 == 0) -> [128, n//16] int16 wrapped + replicated."""
    n = idx.shape[0]
    w = idx.reshape(n // 16, 16).T
    return np.tile(w, (8, 1)).astype(np.int16)


def prepare(inputs):
    """Host-side sharding: per-core padded edge buckets + gather indices."""
    x = np.ascontiguousarray(np.asarray(inputs["x"], dtype=np.float32))
    w_ij = np.ascontiguousarray(np.asarray(inputs["w_ij"], dtype=np.float32))
    seg_i = np.asarray(inputs["seg_i"]).astype(np.int64).ravel()
    idx_j = np.asarray(inputs["idx_j"]).astype(np.int64).ravel()
    W_in = np.ascontiguousarray(np.asarray(inputs["W_in"], dtype=np.float32))
    W_out = np.ascontiguousarray(np.asarray(inputs["W_out"], dtype=np.float32))
    b_out = np.asarray(inputs["b_out"], dtype=np.float32).ravel()

    # edge run boundaries for every 128-atom sub-window of every core
    bounds = []
    for c in range(NCORES):
        for s in range(NSW):
            bounds.append(c * APC + s * SUB)
    bounds.append(NA)
    edges = np.searchsorted(seg_i, np.asarray(bounds, dtype=np.int64))

    # per-sub-window lo/hi (by idx_j half) counts -> global chunk capacities
    nsw_tot = NCORES * NSW
    lo_masks = []
    n_lo = np.zeros(nsw_tot, dtype=np.int64)
    n_hi = np.zeros(nsw_tot, dtype=np.int64)
    for k in range(nsw_tot):
        lo, hi = edges[k], edges[k + 1]
        m = idx_j[lo:hi] < HALF
        lo_masks.append(m)
        n_lo[k] = int(m.sum())
        n_hi[k] = int((hi - lo) - n_lo[k])
    cap_lo = max(1, int(-(-n_lo.max() // P)))
    cap_hi = max(1, int(-(-n_hi.max() // P)))
    cap = cap_lo + cap_hi
    esw = cap * P

    iota_t = np.tile(np.arange(P, dtype=np.float32), (P, cap))
    bias_t = np.tile(b_out[None, :], (P, 1)).astype(np.float32)
    xT = np.ascontiguousarray(x.T)

    in_maps = []
    for c in range(NCORES):
        wdev = np.zeros((NSW, P, esw), dtype=np.float32)
        segw = np.zeros((P, NSW * cap), dtype=np.float32)
        idx16 = np.zeros((P, NSW * cap * 8), dtype=np.int16)
        for s in range(NSW):
            k = c * NSW + s
            lo, hi = edges[k], edges[k + 1]
            if hi == lo:
                continue
            m = lo_masks[k]
            e_idx = idx_j[lo:hi]
            e_seg = (seg_i[lo:hi] - (c * APC + s * SUB)).astype(np.float32)
            e_w = w_ij[lo:hi]
            nl = int(n_lo[k])
            nh = int(n_hi[k])

            wpad = np.zeros((esw, D), dtype=np.float32)
            spad = np.zeros(esw, dtype=np.float32)
            ilo = np.zeros(cap_lo * P, dtype=np.int16)
            ihi = np.zeros(cap_hi * P, dtype=np.int16)

            wpad[:nl] = e_w[m]
            spad[:nl] = e_seg[m]
            ilo[:nl] = e_idx[m].astype(np.int16)
            base = cap_lo * P
            wpad[base : base + nh] = e_w[~m]
            spad[base : base + nh] = e_seg[~m]
            ihi[:nh] = (e_idx[~m] - HALF).astype(np.int16)

            wdev[s] = wpad.reshape(cap, P, D).transpose(1, 0, 2).reshape(P, esw)
            segw[:, s * cap : (s + 1) * cap] = spad.reshape(cap, P).T
            idx16[:, s * cap * 8 : s * cap * 8 + cap_lo * 8] = _wrap_idx(ilo)
            idx16[:, s * cap * 8 + cap_lo * 8 : (s + 1) * cap * 8] = _wrap_idx(ihi)
        in_maps.append(
            {
                "xT": xT,
                "wdev": wdev,
                "segw": segw,
                "idx16": idx16,
                "iota": iota_t,
                "Win": W_in,
                "Wout": W_out,
                "bias": bias_t,
            }
        )
    return cap_lo, cap_hi, in_maps


def kernel(**inputs) -> np.ndarray:
    from concourse.bass_utils import run_bass_kernel_spmd

    cap_lo, cap_hi, in_maps = prepare(inputs)
    nc = build_program(cap_lo, cap_hi)
    nc.finalize()
    res = run_bass_kernel_spmd(nc, in_maps, core_ids=list(range(NCORES)))
    return np.concatenate([r["out"] for r in res.results], axis=0)


# revision 14
# speedup vs baseline: 1.0010x; 1.0010x over previous
"""CFConv (SchNet continuous-filter conv) Trainium2 Bass kernel, 8-core SPMD.

Reference computation:
    f    = x @ W_in                        # (40000, 128)
    f_j  = f[idx_j]                        # (640000, 128) gather
    wf   = w_ij * f_j                      # elementwise
    conv = segment_sum(wf, seg_i, 40000)   # seg_i sorted
    out  = conv @ W_out + b_out

Sharding: seg_i is sorted, so atoms are sharded into 8 contiguous ranges of
5000 and each core gets the contiguous run of edges whose seg_i falls in its
range (found with searchsorted on the host).  No collective is needed: each
core owns its 5000 output rows.

Per core the edge run is re-bucketed by 128-atom sub-window of seg_i, each
sub-window padded to a fixed chunk capacity so all 8 cores run one identical
SPMD program.  Because dma_gather indices are int16, each sub-window's edges
are split by idx_j half (< 20000 vs >= 20000) into leading / trailing chunk
groups and gathered by two dma_gather calls (the second from an offset AP of
the f scratch).  On device:

  phase 1: f = x @ W_in into an HBM scratch (x passed pre-transposed so x
           tiles serve directly as matmul lhsT).
  phase 2: per sub-window: DMA the wf-ready w tile, dma_gather f[idx_j] rows,
           DVE multiply, build the one-hot segment matrix with an is_equal
           compare against an iota tile, and matmul-accumulate
           convT[feat, atom] in PSUM (contraction over the edge partition
           axis).  Per 1024-atom window: fac2out matmul with W_out + bias.
"""

import numpy as np

import concourse.bass as bass
import concourse.mybir as mybir
from concourse import bacc
from concourse.tile import TileContext

P = 128
NA = 40000          # atoms
NE = 640000         # edges
D = 128             # feature dim (FAN_IN == NFM == FAN_OUT)
HALF = NA // 2      # dma_gather int16 index limit workaround
NCORES = 8
APC = NA // NCORES  # atoms per core = 5000
WIN = 1024          # atoms per PSUM window (2 banks)
SUB = 128           # atoms per sub-window (one matmul N slice)
NSW = (APC + SUB - 1) // SUB   # sub-windows per core = 40

F32 = mybir.dt.float32
I16 = mybir.dt.int16


def build_program(cap_lo: int, cap_hi: int):
    """One SPMD program, identical across cores."""
    nc = bacc.Bacc(None, target_bir_lowering=False, debug=False)
    cap = cap_lo + cap_hi
    esw = cap * P

    xT_h = nc.dram_tensor("xT", [P, NA], F32, kind="ExternalInput")
    wdev_h = nc.dram_tensor("wdev", [NSW, P, esw], F32, kind="ExternalInput")
    segw_h = nc.dram_tensor("segw", [P, NSW * cap], F32, kind="ExternalInput")
    idx16_h = nc.dram_tensor("idx16", [P, NSW * cap * 8], I16, kind="ExternalInput")
    gcnt_h = nc.dram_tensor("gcnt", [1, P], mybir.dt.int32, kind="ExternalInput")
    iota_h = nc.dram_tensor("iota", [P, esw], F32, kind="ExternalInput")
    win_h = nc.dram_tensor("Win", [P, P], F32, kind="ExternalInput")
    wout_h = nc.dram_tensor("Wout", [P, P], F32, kind="ExternalInput")
    bias_h = nc.dram_tensor("bias", [P, P], F32, kind="ExternalInput")
    out_h = nc.dram_tensor("out", [APC, D], F32, kind="ExternalOutput")
    f_h = nc.dram_tensor("fscratch", [NA, D], F32, kind="Internal")

    with TileContext(nc) as tc:
        with tc.tile_pool(name="const", bufs=1) as const:
            win_t = const.tile([P, P], F32)
            nc.sync.dma_start(win_t[:], win_h[:, :])
            wout_t = const.tile([P, P], F32)
            nc.sync.dma_start(wout_t[:], wout_h[:, :])
            bias_t = const.tile([P, P], F32)
            nc.sync.dma_start(bias_t[:], bias_h[:, :])
            iota_t = const.tile([P, esw], F32)
            nc.sync.dma_start(iota_t[:], iota_h[:, :])
            segw_t = const.tile([P, NSW * cap], F32)
            nc.sync.dma_start(segw_t[:], segw_h[:, :])
            idx16_t = const.tile([P, NSW * cap * 8], I16)
            nc.sync.dma_start(idx16_t[:], idx16_h[:, :])
            gcnt_t = const.tile([1, P], mybir.dt.int32)
            nc.sync.dma_start(gcnt_t[:], gcnt_h[:, :])

            # ---- phase 1: f = x @ W_in -> HBM scratch ----
            with (
                tc.tile_pool(name="xp", bufs=3) as xp,
                tc.tile_pool(name="fp", bufs=3) as fp,
                tc.tile_pool(name="ps1", bufs=2, space="PSUM") as ps1,
            ):
                a0 = 0
                while a0 < NA:
                    an = min(512, NA - a0)
                    xt = xp.tile([P, 512], F32)
                    nc.sync.dma_start(xt[:, :an], xT_h[:, a0 : a0 + an])
                    fps = ps1.tile([P, 4, P], F32)
                    nt = (an + P - 1) // P
                    for i in range(nt):
                        m = min(P, an - i * P)
                        nc.tensor.matmul(
                            fps[:m, i, :],
                            lhsT=xt[:, i * P : i * P + m],
                            rhs=win_t[:],
                            start=True,
                            stop=True,
                        )
                    fsb = fp.tile([P, 4, P], F32)
                    if an % P == 0:
                        nc.vector.tensor_copy(fsb[:, :nt, :], fps[:, :nt, :])
                        nc.sync.dma_start(
                            f_h[a0 : a0 + an, :].rearrange("(i p) e -> p i e", p=P),
                            fsb[:, :nt, :],
                        )
                    else:
                        nc.vector.tensor_copy(fsb[:an, 0, :], fps[:an, 0, :])
                        nc.sync.dma_start(f_h[a0 : a0 + an, :], fsb[:an, 0, :])
                    a0 += an

            # ---- phase 2: gather, multiply, segment-sum, fac2out ----
            with (
                tc.tile_pool(name="wp", bufs=3) as wp,
                tc.tile_pool(name="fjp", bufs=3) as fjp,
                tc.tile_pool(name="ohp", bufs=2) as ohp,
                tc.tile_pool(name="cvp", bufs=2) as cvp,
                tc.tile_pool(name="owp", bufs=2) as owp,
                tc.tile_pool(name="ps2", bufs=2, space="PSUM") as ps2,
                tc.tile_pool(name="ps3", bufs=2, space="PSUM") as ps3,
            ):
                psT = None
                rcnt_lo = nc.gpsimd.alloc_register("gcnt_lo")
                rcnt_hi = nc.gpsimd.alloc_register("gcnt_hi")
                for s in range(NSW):
                    w_i, sl = divmod(s, WIN // SUB)
                    wt = wp.tile([P, cap, P], F32)
                    nc.sync.dma_start(
                        wt[:], wdev_h[s].rearrange("p (c e) -> p c e", e=P)
                    )
                    fj = fjp.tile([P, cap, P], F32)
                    ib = s * cap * 8
                    # Per-core real index counts come from gcnt; trailing -1
                    # pads emit no descriptors, so pad slots must be zeroed
                    # first (ACT engine is otherwise idle).  single_packet=
                    # False: >1008 idxs exceeds the 64-desc packet ceiling
                    # (HW-verified INTERNAL error otherwise).
                    nc.scalar.memzero(fj[:])
                    nc.gpsimd.reg_load(rcnt_lo, gcnt_t[0:1, 2 * s : 2 * s + 1])
                    nc.gpsimd.dma_gather(
                        fj[:, 0:cap_lo, :],
                        f_h[0:HALF, :],
                        idx16_t[:, ib : ib + cap_lo * 8],
                        cap_lo * P,
                        rcnt_lo,
                        D,
                        single_packet=False,
                    )
                    nc.gpsimd.reg_load(rcnt_hi, gcnt_t[0:1, 2 * s + 1 : 2 * s + 2])
                    nc.gpsimd.dma_gather(
                        fj[:, cap_lo:cap, :],
                        f_h[HALF:NA, :],
                        idx16_t[:, ib + cap_lo * 8 : ib + cap * 8],
                        cap_hi * P,
                        rcnt_hi,
                        D,
                        single_packet=False,
                    )
                    nc.vector.tensor_mul(wt[:], wt[:], fj[:])
                    oh = ohp.tile([P, cap, P], F32)
                    nc.vector.tensor_tensor(
                        out=oh[:],
                        in0=segw_t[:, s * cap : (s + 1) * cap]
                        .unsqueeze(2)
                        .to_broadcast([P, cap, P]),
                        in1=iota_t[:].rearrange("p (c e) -> p c e", e=P),
                        op=mybir.AluOpType.is_equal,
                    )
                    if sl == 0:
                        psT = ps2.tile([P, WIN], F32)
                    for ch in range(cap):
                        nc.tensor.matmul(
                            psT[:, sl * SUB : (sl + 1) * SUB],
                            lhsT=wt[:, ch, :],
                            rhs=oh[:, ch, :],
                            start=(ch == 0),
                            stop=(ch == cap - 1),
                        )
                    if sl == WIN // SUB - 1 or s == NSW - 1:
                        wa0 = w_i * WIN
                        wan = min(WIN, APC - wa0)
                        cvt = cvp.tile([P, WIN], F32)
                        nc.vector.tensor_copy(cvt[:], psT[:])
                        ow = owp.tile([P, WIN // SUB, P], F32)
                        nblk = (wan + P - 1) // P
                        for b in range(nblk):
                            bm = min(P, wan - b * P)
                            ops3 = ps3.tile([P, P], F32)
                            nc.tensor.matmul(
                                ops3[:bm, :],
                                lhsT=cvt[:, b * P : b * P + bm],
                                rhs=wout_t[:],
                                start=True,
                                stop=True,
                            )
                            nc.vector.tensor_add(
                                ow[:bm, b, :], ops3[:bm, :], bias_t[:bm, :]
                            )
                        nfull = wan // P
                        if nfull:
                            nc.sync.dma_start(
                                out_h[wa0 : wa0 + nfull * P, :].rearrange(
                                    "(b p) e -> p b e", p=P
                                ),
                                ow[:, :nfull, :],
                            )
                        rem = wan - nfull * P
                        if rem:
                            nc.sync.dma_start(
                                out_h[wa0 + nfull * P : wa0 + wan, :],
                                ow[:rem, nfull, :],
                            )
    return nc


def _wrap_idx(idx):
    """idx [n] (n % 128 == 0) -> [128, n//16] int16 wrapped + replicated."""
    n = idx.shape[0]
    w = idx.reshape(n // 16, 16).T
    return np.tile(w, (8, 1)).astype(np.int16)


def prepare(inputs):
    """Host-side sharding: per-core padded edge buckets + gather indices."""
    x = np.ascontiguousarray(np.asarray(inputs["x"], dtype=np.float32))
    w_ij = np.ascontiguousarray(np.asarray(inputs["w_ij"], dtype=np.float32))
    seg_i = np.asarray(inputs["seg_i"]).astype(np.int64).ravel()
    idx_j = np.asarray(inputs["idx_j"]).astype(np.int64).ravel()
    W_in = np.ascontiguousarray(np.asarray(inputs["W_in"], dtype=np.float32))
    W_out = np.ascontiguousarray(np.asarray(inputs["W_out"], dtype=np.float32))
    b_out = np.asarray(inputs["b_out"], dtype=np.float32).ravel()

    # edge run boundaries for every 128-atom sub-window of every core
    bounds = []
    for c in range(NCORES):
        for s in range(NSW):
            bounds.append(c * APC + s * SUB)
    bounds.append(NA)
    edges = np.searchsorted(seg_i, np.asarray(bounds, dtype=np.int64))

    # per-sub-window lo/hi (by idx_j half) counts -> global chunk capacities
    nsw_tot = NCORES * NSW
    lo_masks = []
    n_lo = np.zeros(nsw_tot, dtype=np.int64)
    n_hi = np.zeros(nsw_tot, dtype=np.int64)
    for k in range(nsw_tot):
        lo, hi = edges[k], edges[k + 1]
        m = idx_j[lo:hi] < HALF
        lo_masks.append(m)
        n_lo[k] = int(m.sum())
        n_hi[k] = int((hi - lo) - n_lo[k])
    cap_lo = max(1, int(-(-n_lo.max() // P)))
    cap_hi = max(1, int(-(-n_hi.max() // P)))
    cap = cap_lo + cap_hi
    esw = cap * P

    iota_t = np.tile(np.arange(P, dtype=np.float32), (P, cap))
    bias_t = np.tile(b_out[None, :], (P, 1)).astype(np.float32)
    xT = np.ascontiguousarray(x.T)

    in_maps = []
    for c in range(NCORES):
        wdev = np.zeros((NSW, P, esw), dtype=np.float32)
        segw = np.zeros((P, NSW * cap), dtype=np.float32)
        idx16 = np.zeros((P, NSW * cap * 8), dtype=np.int16)
        gcnt = np.zeros((1, P), dtype=np.int32)
        for s in range(NSW):
            k = c * NSW + s
            lo, hi = edges[k], edges[k + 1]
            m = lo_masks[k]
            e_idx = idx_j[lo:hi]
            e_seg = (seg_i[lo:hi] - (c * APC + s * SUB)).astype(np.float32)
            e_w = w_ij[lo:hi]
            nl = int(n_lo[k])
            nh = int(n_hi[k])

            wpad = np.zeros((esw, D), dtype=np.float32)
            spad = np.zeros(esw, dtype=np.float32)
            ilo = np.full(cap_lo * P, -1, dtype=np.int16)
            ihi = np.full(cap_hi * P, -1, dtype=np.int16)

            wpad[:nl] = e_w[m]
            spad[:nl] = e_seg[m]
            ilo[:nl] = e_idx[m].astype(np.int16)
            base = cap_lo * P
            wpad[base : base + nh] = e_w[~m]
            spad[base : base + nh] = e_seg[~m]
            ihi[:nh] = (e_idx[~m] - HALF).astype(np.int16)
            if nl == 0:
                ilo[0] = 0
            if nh == 0:
                ihi[0] = 0
            gcnt[0, 2 * s] = max(1, nl)
            gcnt[0, 2 * s + 1] = max(1, nh)

            wdev[s] = wpad.reshape(cap, P, D).transpose(1, 0, 2).reshape(P, esw)
            segw[:, s * cap : (s + 1) * cap] = spad.reshape(cap, P).T
            idx16[:, s * cap * 8 : s * cap * 8 + cap_lo * 8] = _wrap_idx(ilo)
            idx16[:, s * cap * 8 + cap_lo * 8 : (s + 1) * cap * 8] = _wrap_idx(ihi)
        in_maps.append(
            {
                "xT": xT,
                "wdev": wdev,
                "segw": segw,
                "idx16": idx16,
                "gcnt": gcnt,
                "iota": iota_t,
                "Win": W_in,
                "Wout": W_out,
                "bias": bias_t,
            }
        )
    return cap_lo, cap_hi, in_maps


def kernel(**inputs) -> np.ndarray:
    from concourse.bass_utils import run_bass_kernel_spmd

    cap_lo, cap_hi, in_maps = prepare(inputs)
    nc = build_program(cap_lo, cap_hi)
    nc.finalize()
    res = run_bass_kernel_spmd(nc, in_maps, core_ids=list(range(NCORES)))
    return np.concatenate([r["out"] for r in res.results], axis=0)


# revision 15
# speedup vs baseline: 1.0209x; 1.0199x over previous
"""CFConv (SchNet continuous-filter conv) Trainium2 Bass kernel, 8-core SPMD.

Reference computation:
    f    = x @ W_in                        # (40000, 128)
    f_j  = f[idx_j]                        # (640000, 128) gather
    wf   = w_ij * f_j                      # elementwise
    conv = segment_sum(wf, seg_i, 40000)   # seg_i sorted
    out  = conv @ W_out + b_out

Sharding: seg_i is sorted, so atoms are sharded into 8 contiguous ranges of
5000 and each core gets the contiguous run of edges whose seg_i falls in its
range (found with searchsorted on the host).  No collective is needed: each
core owns its 5000 output rows.

Per core the edge run is re-bucketed by 128-atom sub-window of seg_i, each
sub-window padded to a fixed chunk capacity so all 8 cores run one identical
SPMD program.  Because dma_gather indices are int16, each sub-window's edges
are split by idx_j half (< 20000 vs >= 20000) into leading / trailing chunk
groups and gathered by two dma_gather calls (the second from an offset AP of
the f scratch).  On device:

  phase 1: f = x @ W_in into an HBM scratch (x passed pre-transposed so x
           tiles serve directly as matmul lhsT).
  phase 2: per sub-window: DMA the wf-ready w tile, dma_gather f[idx_j] rows,
           DVE multiply, build the one-hot segment matrix with an is_equal
           compare against an iota tile, and matmul-accumulate
           convT[feat, atom] in PSUM (contraction over the edge partition
           axis).  Per 1024-atom window: fac2out matmul with W_out + bias.
"""

import numpy as np

import concourse.bass as bass
import concourse.mybir as mybir
from concourse import bacc
from concourse.tile import TileContext

P = 128
NA = 40000          # atoms
NE = 640000         # edges
D = 128             # feature dim (FAN_IN == NFM == FAN_OUT)
HALF = NA // 2      # dma_gather int16 index limit workaround
NCORES = 8
APC = NA // NCORES  # atoms per core = 5000
WIN = 1024          # atoms per PSUM window (2 banks)
SUB = 128           # atoms per sub-window (one matmul N slice)
NSW = (APC + SUB - 1) // SUB   # sub-windows per core = 40

F32 = mybir.dt.float32
I16 = mybir.dt.int16


def build_program(cap_lo: int, cap_hi: int):
    """One SPMD program, identical across cores."""
    nc = bacc.Bacc(None, target_bir_lowering=False, debug=False)
    cap = cap_lo + cap_hi
    esw = cap * P

    xT_h = nc.dram_tensor("xT", [P, NA], F32, kind="ExternalInput")
    wdev_h = nc.dram_tensor("wdev", [NSW, P, esw], F32, kind="ExternalInput")
    segw_h = nc.dram_tensor("segw", [P, NSW * cap], F32, kind="ExternalInput")
    idx16_h = nc.dram_tensor("idx16", [P, NSW * cap * 8], I16, kind="ExternalInput")
    gcnt_h = nc.dram_tensor("gcnt", [1, P], mybir.dt.int32, kind="ExternalInput")
    iota_h = nc.dram_tensor("iota", [P, esw], F32, kind="ExternalInput")
    win_h = nc.dram_tensor("Win", [P, P], F32, kind="ExternalInput")
    wout_h = nc.dram_tensor("Wout", [P, P], F32, kind="ExternalInput")
    bias_h = nc.dram_tensor("bias", [P, P], F32, kind="ExternalInput")
    out_h = nc.dram_tensor("out", [APC, D], F32, kind="ExternalOutput")
    f_h = nc.dram_tensor("fscratch", [NA, D], F32, kind="Internal")

    with TileContext(nc) as tc:
        with tc.tile_pool(name="const", bufs=1) as const:
            win_t = const.tile([P, P], F32)
            nc.sync.dma_start(win_t[:], win_h[:, :])
            wout_t = const.tile([P, P], F32)
            nc.sync.dma_start(wout_t[:], wout_h[:, :])
            bias_t = const.tile([P, P], F32)
            nc.sync.dma_start(bias_t[:], bias_h[:, :])
            iota_t = const.tile([P, esw], F32)
            nc.sync.dma_start(iota_t[:], iota_h[:, :])
            segw_t = const.tile([P, NSW * cap], F32)
            nc.sync.dma_start(segw_t[:], segw_h[:, :])
            idx16_t = const.tile([P, NSW * cap * 8], I16)
            nc.sync.dma_start(idx16_t[:], idx16_h[:, :])
            gcnt_t = const.tile([1, P], mybir.dt.int32)
            nc.sync.dma_start(gcnt_t[:], gcnt_h[:, :])

            # ---- phase 1: f = x @ W_in -> HBM scratch ----
            with (
                tc.tile_pool(name="xp", bufs=3) as xp,
                tc.tile_pool(name="fp", bufs=3) as fp,
                tc.tile_pool(name="ps1", bufs=2, space="PSUM") as ps1,
            ):
                a0 = 0
                while a0 < NA:
                    an = min(512, NA - a0)
                    xt = xp.tile([P, 512], F32)
                    nc.sync.dma_start(xt[:, :an], xT_h[:, a0 : a0 + an])
                    fps = ps1.tile([P, 4, P], F32)
                    nt = (an + P - 1) // P
                    for i in range(nt):
                        m = min(P, an - i * P)
                        nc.tensor.matmul(
                            fps[:m, i, :],
                            lhsT=xt[:, i * P : i * P + m],
                            rhs=win_t[:],
                            start=True,
                            stop=True,
                        )
                    fsb = fp.tile([P, 4, P], F32)
                    if an % P == 0:
                        nc.vector.tensor_copy(fsb[:, :nt, :], fps[:, :nt, :])
                        nc.sync.dma_start(
                            f_h[a0 : a0 + an, :].rearrange("(i p) e -> p i e", p=P),
                            fsb[:, :nt, :],
                        )
                    else:
                        nc.vector.tensor_copy(fsb[:an, 0, :], fps[:an, 0, :])
                        nc.sync.dma_start(f_h[a0 : a0 + an, :], fsb[:an, 0, :])
                    a0 += an

            # ---- phase 2: gather, multiply, segment-sum, fac2out ----
            LOOK = 5  # lo-gather lookahead: hides Q7 scan under phase 1 / hi
            with (
                tc.tile_pool(name="wp", bufs=3) as wp,
                tc.tile_pool(name="fjp", bufs=LOOK + 2) as fjp,
                tc.tile_pool(name="ohp", bufs=2) as ohp,
                tc.tile_pool(name="cvp", bufs=2) as cvp,
                tc.tile_pool(name="owp", bufs=2) as owp,
                tc.tile_pool(name="ps2", bufs=2, space="PSUM") as ps2,
                tc.tile_pool(name="ps3", bufs=2, space="PSUM") as ps3,
            ):
                psT = None
                rcnt_lo = nc.gpsimd.alloc_register("gcnt_lo")
                rcnt_hi = nc.gpsimd.alloc_register("gcnt_hi")
                fj_q = {}

                def emit_lo(s):
                    # Per-core real index counts come from gcnt; trailing -1
                    # pads emit no descriptors, so pad slots must be zeroed
                    # first (ACT engine is otherwise idle).  single_packet=
                    # False: >1008 idxs exceeds the 64-desc packet ceiling
                    # (HW-verified INTERNAL error otherwise).
                    fj = fjp.tile([P, cap, P], F32, tag="fj")
                    nc.scalar.memzero(fj[:])
                    nc.gpsimd.reg_load(rcnt_lo, gcnt_t[0:1, 2 * s : 2 * s + 1])
                    nc.gpsimd.dma_gather(
                        fj[:, 0:cap_lo, :],
                        f_h[0:HALF, :],
                        idx16_t[:, s * cap * 8 : s * cap * 8 + cap_lo * 8],
                        cap_lo * P,
                        rcnt_lo,
                        D,
                        single_packet=False,
                    )
                    fj_q[s] = fj

                for s in range(min(LOOK, NSW)):
                    emit_lo(s)
                for s in range(NSW):
                    w_i, sl = divmod(s, WIN // SUB)
                    wt = wp.tile([P, cap, P], F32)
                    nc.sync.dma_start(
                        wt[:], wdev_h[s].rearrange("p (c e) -> p c e", e=P)
                    )
                    fj = fj_q.pop(s)
                    ib = s * cap * 8
                    nc.gpsimd.reg_load(rcnt_hi, gcnt_t[0:1, 2 * s + 1 : 2 * s + 2])
                    nc.gpsimd.dma_gather(
                        fj[:, cap_lo:cap, :],
                        f_h[HALF:NA, :],
                        idx16_t[:, ib + cap_lo * 8 : ib + cap * 8],
                        cap_hi * P,
                        rcnt_hi,
                        D,
                        single_packet=False,
                    )
                    if s + LOOK < NSW:
                        emit_lo(s + LOOK)
                    nc.vector.tensor_mul(wt[:], wt[:], fj[:])
                    oh = ohp.tile([P, cap, P], F32)
                    nc.vector.tensor_tensor(
                        out=oh[:],
                        in0=segw_t[:, s * cap : (s + 1) * cap]
                        .unsqueeze(2)
                        .to_broadcast([P, cap, P]),
                        in1=iota_t[:].rearrange("p (c e) -> p c e", e=P),
                        op=mybir.AluOpType.is_equal,
                    )
                    if sl == 0:
                        psT = ps2.tile([P, WIN], F32)
                    for ch in range(cap):
                        nc.tensor.matmul(
                            psT[:, sl * SUB : (sl + 1) * SUB],
                            lhsT=wt[:, ch, :],
                            rhs=oh[:, ch, :],
                            start=(ch == 0),
                            stop=(ch == cap - 1),
                        )
                    if sl == WIN // SUB - 1 or s == NSW - 1:
                        wa0 = w_i * WIN
                        wan = min(WIN, APC - wa0)
                        cvt = cvp.tile([P, WIN], F32)
                        nc.vector.tensor_copy(cvt[:], psT[:])
                        ow = owp.tile([P, WIN // SUB, P], F32)
                        nblk = (wan + P - 1) // P
                        for b in range(nblk):
                            bm = min(P, wan - b * P)
                            ops3 = ps3.tile([P, P], F32)
                            nc.tensor.matmul(
                                ops3[:bm, :],
                                lhsT=cvt[:, b * P : b * P + bm],
                                rhs=wout_t[:],
                                start=True,
                                stop=True,
                            )
                            nc.vector.tensor_add(
                                ow[:bm, b, :], ops3[:bm, :], bias_t[:bm, :]
                            )
                        nfull = wan // P
                        if nfull:
                            nc.sync.dma_start(
                                out_h[wa0 : wa0 + nfull * P, :].rearrange(
                                    "(b p) e -> p b e", p=P
                                ),
                                ow[:, :nfull, :],
                            )
                        rem = wan - nfull * P
                        if rem:
                            nc.sync.dma_start(
                                out_h[wa0 + nfull * P : wa0 + wan, :],
                                ow[:rem, nfull, :],
                            )
    return nc


def _wrap_idx(idx):
    """idx [n] (n % 128 == 0) -> [128, n//16] int16 wrapped + replicated."""
    n = idx.shape[0]
    w = idx.reshape(n // 16, 16).T
    return np.tile(w, (8, 1)).astype(np.int16)


def prepare(inputs):
    """Host-side sharding: per-core padded edge buckets + gather indices."""
    x = np.ascontiguousarray(np.asarray(inputs["x"], dtype=np.float32))
    w_ij = np.ascontiguousarray(np.asarray(inputs["w_ij"], dtype=np.float32))
    seg_i = np.asarray(inputs["seg_i"]).astype(np.int64).ravel()
    idx_j = np.asarray(inputs["idx_j"]).astype(np.int64).ravel()
    W_in = np.ascontiguousarray(np.asarray(inputs["W_in"], dtype=np.float32))
    W_out = np.ascontiguousarray(np.asarray(inputs["W_out"], dtype=np.float32))
    b_out = np.asarray(inputs["b_out"], dtype=np.float32).ravel()

    # edge run boundaries for every 128-atom sub-window of every core
    bounds = []
    for c in range(NCORES):
        for s in range(NSW):
            bounds.append(c * APC + s * SUB)
    bounds.append(NA)
    edges = np.searchsorted(seg_i, np.asarray(bounds, dtype=np.int64))

    # per-sub-window lo/hi (by idx_j half) counts -> global chunk capacities
    nsw_tot = NCORES * NSW
    lo_masks = []
    n_lo = np.zeros(nsw_tot, dtype=np.int64)
    n_hi = np.zeros(nsw_tot, dtype=np.int64)
    for k in range(nsw_tot):
        lo, hi = edges[k], edges[k + 1]
        m = idx_j[lo:hi] < HALF
        lo_masks.append(m)
        n_lo[k] = int(m.sum())
        n_hi[k] = int((hi - lo) - n_lo[k])
    cap_lo = max(1, int(-(-n_lo.max() // P)))
    cap_hi = max(1, int(-(-n_hi.max() // P)))
    cap = cap_lo + cap_hi
    esw = cap * P

    iota_t = np.tile(np.arange(P, dtype=np.float32), (P, cap))
    bias_t = np.tile(b_out[None, :], (P, 1)).astype(np.float32)
    xT = np.ascontiguousarray(x.T)

    in_maps = []
    for c in range(NCORES):
        wdev = np.zeros((NSW, P, esw), dtype=np.float32)
        segw = np.zeros((P, NSW * cap), dtype=np.float32)
        idx16 = np.zeros((P, NSW * cap * 8), dtype=np.int16)
        gcnt = np.zeros((1, P), dtype=np.int32)
        for s in range(NSW):
            k = c * NSW + s
            lo, hi = edges[k], edges[k + 1]
            m = lo_masks[k]
            e_idx = idx_j[lo:hi]
            e_seg = (seg_i[lo:hi] - (c * APC + s * SUB)).astype(np.float32)
            e_w = w_ij[lo:hi]
            nl = int(n_lo[k])
            nh = int(n_hi[k])

            wpad = np.zeros((esw, D), dtype=np.float32)
            spad = np.zeros(esw, dtype=np.float32)
            ilo = np.full(cap_lo * P, -1, dtype=np.int16)
            ihi = np.full(cap_hi * P, -1, dtype=np.int16)

            wpad[:nl] = e_w[m]
            spad[:nl] = e_seg[m]
            ilo[:nl] = e_idx[m].astype(np.int16)
            base = cap_lo * P
            wpad[base : base + nh] = e_w[~m]
            spad[base : base + nh] = e_seg[~m]
            ihi[:nh] = (e_idx[~m] - HALF).astype(np.int16)
            if nl == 0:
                ilo[0] = 0
            if nh == 0:
                ihi[0] = 0
            gcnt[0, 2 * s] = max(1, nl)
            gcnt[0, 2 * s + 1] = max(1, nh)

            wdev[s] = wpad.reshape(cap, P, D).transpose(1, 0, 2).reshape(P, esw)
            segw[:, s * cap : (s + 1) * cap] = spad.reshape(cap, P).T
            idx16[:, s * cap * 8 : s * cap * 8 + cap_lo * 8] = _wrap_idx(ilo)
            idx16[:, s * cap * 8 + cap_lo * 8 : (s + 1) * cap * 8] = _wrap_idx(ihi)
        in_maps.append(
            {
                "xT": xT,
                "wdev": wdev,
                "segw": segw,
                "idx16": idx16,
                "gcnt": gcnt,
                "iota": iota_t,
                "Win": W_in,
                "Wout": W_out,
                "bias": bias_t,
            }
        )
    return cap_lo, cap_hi, in_maps


def kernel(**inputs) -> np.ndarray:
    from concourse.bass_utils import run_bass_kernel_spmd

    cap_lo, cap_hi, in_maps = prepare(inputs)
    nc = build_program(cap_lo, cap_hi)
    nc.finalize()
    res = run_bass_kernel_spmd(nc, in_maps, core_ids=list(range(NCORES)))
    return np.concatenate([r["out"] for r in res.results], axis=0)


# revision 18
# speedup vs baseline: 1.0215x; 1.0006x over previous
"""CFConv (SchNet continuous-filter conv) Trainium2 Bass kernel, 8-core SPMD.

Reference computation:
    f    = x @ W_in                        # (40000, 128)
    f_j  = f[idx_j]                        # (640000, 128) gather
    wf   = w_ij * f_j                      # elementwise
    conv = segment_sum(wf, seg_i, 40000)   # seg_i sorted
    out  = conv @ W_out + b_out

Sharding: seg_i is sorted, so atoms are sharded into 8 contiguous ranges of
5000 and each core gets the contiguous run of edges whose seg_i falls in its
range (found with searchsorted on the host).  No collective is needed: each
core owns its 5000 output rows.

Per core the edge run is re-bucketed by 128-atom sub-window of seg_i, each
sub-window padded to a fixed chunk capacity so all 8 cores run one identical
SPMD program.  Because dma_gather indices are int16, each sub-window's edges
are split by idx_j half (< 20000 vs >= 20000) into leading / trailing chunk
groups and gathered by two dma_gather calls (the second from an offset AP of
the f scratch).  On device:

  phase 1: f = x @ W_in into an HBM scratch (x passed pre-transposed so x
           tiles serve directly as matmul lhsT).
  phase 2: per sub-window: DMA the wf-ready w tile, dma_gather f[idx_j] rows,
           DVE multiply, build the one-hot segment matrix with an is_equal
           compare against an iota tile, and matmul-accumulate
           convT[feat, atom] in PSUM (contraction over the edge partition
           axis).  Per 1024-atom window: fac2out matmul with W_out + bias.
"""

import numpy as np

import concourse.bass as bass
import concourse.mybir as mybir
from concourse import bacc
from concourse.tile import TileContext

P = 128
NA = 40000          # atoms
NE = 640000         # edges
D = 128             # feature dim (FAN_IN == NFM == FAN_OUT)
HALF = NA // 2      # dma_gather int16 index limit workaround
NCORES = 8
APC = NA // NCORES  # atoms per core = 5000
WIN = 1024          # atoms per PSUM window (2 banks)
SUB = 128           # atoms per sub-window (one matmul N slice)
NSW = (APC + SUB - 1) // SUB   # sub-windows per core = 40

F32 = mybir.dt.float32
I16 = mybir.dt.int16


def build_program(cap_lo: int, cap_hi: int):
    """One SPMD program, identical across cores."""
    nc = bacc.Bacc(None, target_bir_lowering=False, debug=False)
    cap = cap_lo + cap_hi
    esw = cap * P

    xT_h = nc.dram_tensor("xT", [P, NA], F32, kind="ExternalInput")
    wdev_h = nc.dram_tensor("wdev", [NSW, P, esw], F32, kind="ExternalInput")
    segw_h = nc.dram_tensor("segw", [P, NSW * cap], F32, kind="ExternalInput")
    idx16_h = nc.dram_tensor("idx16", [P, NSW * cap * 8], I16, kind="ExternalInput")
    gcnt_h = nc.dram_tensor("gcnt", [1, P], mybir.dt.int32, kind="ExternalInput")
    iota_h = nc.dram_tensor("iota", [P, esw], F32, kind="ExternalInput")
    win_h = nc.dram_tensor("Win", [P, P], F32, kind="ExternalInput")
    wout_h = nc.dram_tensor("Wout", [P, P], F32, kind="ExternalInput")
    bias_h = nc.dram_tensor("bias", [P, P], F32, kind="ExternalInput")
    out_h = nc.dram_tensor("out", [APC, D], F32, kind="ExternalOutput")
    # two tensors so lo-gathers only dep on the first half of phase 1
    flo_h = nc.dram_tensor("fscratch_lo", [HALF, D], F32, kind="Internal")
    fhi_h = nc.dram_tensor("fscratch_hi", [NA - HALF, D], F32, kind="Internal")

    with TileContext(nc) as tc:
        with tc.tile_pool(name="const", bufs=1) as const:
            win_t = const.tile([P, P], F32)
            nc.sync.dma_start(win_t[:], win_h[:, :])
            wout_t = const.tile([P, P], F32)
            nc.sync.dma_start(wout_t[:], wout_h[:, :])
            bias_t = const.tile([P, P], F32)
            nc.sync.dma_start(bias_t[:], bias_h[:, :])
            iota_t = const.tile([P, esw], F32)
            nc.sync.dma_start(iota_t[:], iota_h[:, :])
            segw_t = const.tile([P, NSW * cap], F32)
            nc.sync.dma_start(segw_t[:], segw_h[:, :])
            idx16_t = const.tile([P, NSW * cap * 8], I16)
            nc.sync.dma_start(idx16_t[:], idx16_h[:, :])
            gcnt_t = const.tile([1, P], mybir.dt.int32)
            nc.sync.dma_start(gcnt_t[:], gcnt_h[:, :])

            # ---- phase 1: f = x @ W_in -> HBM scratch ----
            with (
                tc.tile_pool(name="xp", bufs=3) as xp,
                tc.tile_pool(name="fp", bufs=3) as fp,
                tc.tile_pool(name="ps1", bufs=2, space="PSUM") as ps1,
            ):
                for half_h, h0 in ((flo_h, 0), (fhi_h, HALF)):
                    a0 = 0
                    hn = HALF if h0 == 0 else NA - HALF
                    while a0 < hn:
                        an = min(512, hn - a0)
                        xt = xp.tile([P, 512], F32)
                        nc.sync.dma_start(
                            xt[:, :an], xT_h[:, h0 + a0 : h0 + a0 + an]
                        )
                        fps = ps1.tile([P, 4, P], F32)
                        nt = (an + P - 1) // P
                        for i in range(nt):
                            m = min(P, an - i * P)
                            nc.tensor.matmul(
                                fps[:m, i, :],
                                lhsT=xt[:, i * P : i * P + m],
                                rhs=win_t[:],
                                start=True,
                                stop=True,
                            )
                        fsb = fp.tile([P, 4, P], F32)
                        if an % P == 0:
                            nc.vector.tensor_copy(fsb[:, :nt, :], fps[:, :nt, :])
                            nc.sync.dma_start(
                                half_h[a0 : a0 + an, :].rearrange(
                                    "(i p) e -> p i e", p=P
                                ),
                                fsb[:, :nt, :],
                            )
                        else:
                            nc.vector.tensor_copy(fsb[:an, 0, :], fps[:an, 0, :])
                            nc.sync.dma_start(half_h[a0 : a0 + an, :], fsb[:an, 0, :])
                        a0 += an

            # ---- phase 2: gather, multiply, segment-sum, fac2out ----
            LOOK = 5  # lo-gather lookahead: hides Q7 scan under phase 1 / hi
            with (
                tc.tile_pool(name="wp", bufs=3) as wp,
                tc.tile_pool(name="fjp", bufs=LOOK + 2) as fjp,
                tc.tile_pool(name="ohp", bufs=2) as ohp,
                tc.tile_pool(name="cvp", bufs=2) as cvp,
                tc.tile_pool(name="owp", bufs=2) as owp,
                tc.tile_pool(name="ps2", bufs=2, space="PSUM") as ps2,
                tc.tile_pool(name="ps3", bufs=2, space="PSUM") as ps3,
            ):
                psT = None
                rcnt_lo = nc.gpsimd.alloc_register("gcnt_lo")
                rcnt_hi = nc.gpsimd.alloc_register("gcnt_hi")
                fj_q = {}

                def emit_lo(s):
                    # Per-core real index counts come from gcnt; trailing -1
                    # pads emit no descriptors, so pad slots must be zeroed
                    # first (ACT engine is otherwise idle).  single_packet=
                    # False: >1008 idxs exceeds the 64-desc packet ceiling
                    # (HW-verified INTERNAL error otherwise).
                    fj = fjp.tile([P, cap, P], F32, tag="fj")
                    nc.scalar.memzero(fj[:])
                    nc.gpsimd.reg_load(rcnt_lo, gcnt_t[0:1, 2 * s : 2 * s + 1])
                    nc.gpsimd.dma_gather(
                        fj[:, 0:cap_lo, :],
                        flo_h[:, :],
                        idx16_t[:, s * cap * 8 : s * cap * 8 + cap_lo * 8],
                        cap_lo * P,
                        rcnt_lo,
                        D,
                        single_packet=False,
                    )
                    fj_q[s] = fj

                for s in range(min(LOOK, NSW)):
                    emit_lo(s)
                for s in range(NSW):
                    w_i, sl = divmod(s, WIN // SUB)
                    wt = wp.tile([P, cap, P], F32)
                    nc.sync.dma_start(
                        wt[:], wdev_h[s].rearrange("p (c e) -> p c e", e=P)
                    )
                    fj = fj_q.pop(s)
                    ib = s * cap * 8
                    nc.gpsimd.reg_load(rcnt_hi, gcnt_t[0:1, 2 * s + 1 : 2 * s + 2])
                    nc.gpsimd.dma_gather(
                        fj[:, cap_lo:cap, :],
                        fhi_h[:, :],
                        idx16_t[:, ib + cap_lo * 8 : ib + cap * 8],
                        cap_hi * P,
                        rcnt_hi,
                        D,
                        single_packet=False,
                    )
                    if s + LOOK < NSW:
                        emit_lo(s + LOOK)
                    nc.vector.tensor_mul(wt[:], wt[:], fj[:])
                    oh = ohp.tile([P, cap, P], F32)
                    nc.vector.tensor_tensor(
                        out=oh[:],
                        in0=segw_t[:, s * cap : (s + 1) * cap]
                        .unsqueeze(2)
                        .to_broadcast([P, cap, P]),
                        in1=iota_t[:].rearrange("p (c e) -> p c e", e=P),
                        op=mybir.AluOpType.is_equal,
                    )
                    if sl == 0:
                        psT = ps2.tile([P, WIN], F32)
                    for ch in range(cap):
                        nc.tensor.matmul(
                            psT[:, sl * SUB : (sl + 1) * SUB],
                            lhsT=wt[:, ch, :],
                            rhs=oh[:, ch, :],
                            start=(ch == 0),
                            stop=(ch == cap - 1),
                        )
                    if sl == WIN // SUB - 1 or s == NSW - 1:
                        wa0 = w_i * WIN
                        wan = min(WIN, APC - wa0)
                        cvt = cvp.tile([P, WIN], F32)
                        nc.vector.tensor_copy(cvt[:], psT[:])
                        ow = owp.tile([P, WIN // SUB, P], F32)
                        nblk = (wan + P - 1) // P
                        for b in range(nblk):
                            bm = min(P, wan - b * P)
                            ops3 = ps3.tile([P, P], F32)
                            nc.tensor.matmul(
                                ops3[:bm, :],
                                lhsT=cvt[:, b * P : b * P + bm],
                                rhs=wout_t[:],
                                start=True,
                                stop=True,
                            )
                            nc.vector.tensor_add(
                                ow[:bm, b, :], ops3[:bm, :], bias_t[:bm, :]
                            )
                        nfull = wan // P
                        if nfull:
                            nc.sync.dma_start(
                                out_h[wa0 : wa0 + nfull * P, :].rearrange(
                                    "(b p) e -> p b e", p=P
                                ),
                                ow[:, :nfull, :],
                            )
                        rem = wan - nfull * P
                        if rem:
                            nc.sync.dma_start(
                                out_h[wa0 + nfull * P : wa0 + wan, :],
                                ow[:rem, nfull, :],
                            )
    return nc


def _wrap_idx(idx):
    """idx [n] (n % 128 == 0) -> [128, n//16] int16 wrapped + replicated."""
    n = idx.shape[0]
    w = idx.reshape(n // 16, 16).T
    return np.tile(w, (8, 1)).astype(np.int16)


def prepare(inputs):
    """Host-side sharding: per-core padded edge buckets + gather indices."""
    x = np.ascontiguousarray(np.asarray(inputs["x"], dtype=np.float32))
    w_ij = np.ascontiguousarray(np.asarray(inputs["w_ij"], dtype=np.float32))
    seg_i = np.asarray(inputs["seg_i"]).astype(np.int64).ravel()
    idx_j = np.asarray(inputs["idx_j"]).astype(np.int64).ravel()
    W_in = np.ascontiguousarray(np.asarray(inputs["W_in"], dtype=np.float32))
    W_out = np.ascontiguousarray(np.asarray(inputs["W_out"], dtype=np.float32))
    b_out = np.asarray(inputs["b_out"], dtype=np.float32).ravel()

    # edge run boundaries for every 128-atom sub-window of every core
    bounds = []
    for c in range(NCORES):
        for s in range(NSW):
            bounds.append(c * APC + s * SUB)
    bounds.append(NA)
    edges = np.searchsorted(seg_i, np.asarray(bounds, dtype=np.int64))

    # per-sub-window lo/hi (by idx_j half) counts -> global chunk capacities
    nsw_tot = NCORES * NSW
    lo_masks = []
    n_lo = np.zeros(nsw_tot, dtype=np.int64)
    n_hi = np.zeros(nsw_tot, dtype=np.int64)
    for k in range(nsw_tot):
        lo, hi = edges[k], edges[k + 1]
        m = idx_j[lo:hi] < HALF
        lo_masks.append(m)
        n_lo[k] = int(m.sum())
        n_hi[k] = int((hi - lo) - n_lo[k])
    cap_lo = max(1, int(-(-n_lo.max() // P)))
    cap_hi = max(1, int(-(-n_hi.max() // P)))
    cap = cap_lo + cap_hi
    esw = cap * P

    iota_t = np.tile(np.arange(P, dtype=np.float32), (P, cap))
    bias_t = np.tile(b_out[None, :], (P, 1)).astype(np.float32)
    xT = np.ascontiguousarray(x.T)

    in_maps = []
    for c in range(NCORES):
        wdev = np.zeros((NSW, P, esw), dtype=np.float32)
        segw = np.zeros((P, NSW * cap), dtype=np.float32)
        idx16 = np.zeros((P, NSW * cap * 8), dtype=np.int16)
        gcnt = np.zeros((1, P), dtype=np.int32)
        for s in range(NSW):
            k = c * NSW + s
            lo, hi = edges[k], edges[k + 1]
            m = lo_masks[k]
            e_idx = idx_j[lo:hi]
            e_seg = (seg_i[lo:hi] - (c * APC + s * SUB)).astype(np.float32)
            e_w = w_ij[lo:hi]
            nl = int(n_lo[k])
            nh = int(n_hi[k])

            wpad = np.zeros((esw, D), dtype=np.float32)
            spad = np.zeros(esw, dtype=np.float32)
            ilo = np.full(cap_lo * P, -1, dtype=np.int16)
            ihi = np.full(cap_hi * P, -1, dtype=np.int16)

            wpad[:nl] = e_w[m]
            spad[:nl] = e_seg[m]
            ilo[:nl] = e_idx[m].astype(np.int16)
            base = cap_lo * P
            wpad[base : base + nh] = e_w[~m]
            spad[base : base + nh] = e_seg[~m]
            ihi[:nh] = (e_idx[~m] - HALF).astype(np.int16)
            if nl == 0:
                ilo[0] = 0
            if nh == 0:
                ihi[0] = 0
            gcnt[0, 2 * s] = max(1, nl)
            gcnt[0, 2 * s + 1] = max(1, nh)

            wdev[s] = wpad.reshape(cap, P, D).transpose(1, 0, 2).reshape(P, esw)
            segw[:, s * cap : (s + 1) * cap] = spad.reshape(cap, P).T
            idx16[:, s * cap * 8 : s * cap * 8 + cap_lo * 8] = _wrap_idx(ilo)
            idx16[:, s * cap * 8 + cap_lo * 8 : (s + 1) * cap * 8] = _wrap_idx(ihi)
        in_maps.append(
            {
                "xT": xT,
                "wdev": wdev,
                "segw": segw,
                "idx16": idx16,
                "gcnt": gcnt,
                "iota": iota_t,
                "Win": W_in,
                "Wout": W_out,
                "bias": bias_t,
            }
        )
    return cap_lo, cap_hi, in_maps


def kernel(**inputs) -> np.ndarray:
    from concourse.bass_utils import run_bass_kernel_spmd

    cap_lo, cap_hi, in_maps = prepare(inputs)
    nc = build_program(cap_lo, cap_hi)
    nc.finalize()
    res = run_bass_kernel_spmd(nc, in_maps, core_ids=list(range(NCORES)))
    return np.concatenate([r["out"] for r in res.results], axis=0)
